# revision 2
# baseline (speedup 1.0000x reference)
"""3D bilateral filter (RADIUS=2, 5x5x5 window) on 8 Trainium2 NeuronCores.

Sharding: 8 cores = 2 batches x 4 z-slabs of 32 (halo 2 handled host-side).
Per-core layout: partitions = x (128), free dim = (z_local rows) x (padded y).
Out-of-volume taps are neutralized by padding with a large value BIG chosen so
the range weight exp(-c*(x-BIG)^2 + b) underflows to exactly 0 on the ACT LUT.
x-axis tap shifts are pre-materialized host-side as 5 shifted variants (plus a
second y-parity copy in fp16 mode, keeping DVE reads 4B-aligned for 2x mode).

Per tap on-chip:  D = x - x_shift (DVE), S = D^2 (ACT Square), W = exp(-c*S+b)
(ACT Exp, b = log spatial weight), P = W * x_shift (DVE), then num += P and
den += W via identity-matmul accumulation into PSUM (PE does all adds).
Finally out = num * reciprocal(den) (DVE) and DMA out.
"""

import os
import sys

import numpy as np

for _p in ("/root/.axon_site", "/root/.axon_site/_ro/trn_rl_repo",
           "/root/.axon_site/_ro/pypackages", "/opt/trn_rl_repo"):
    if os.path.isdir(_p) and _p not in sys.path:
        sys.path.append(_p)

import concourse.bacc as bacc
import concourse.mybir as mybir
from concourse.tile import TileContext
from concourse import bass_utils

RADIUS = 2
NTAPS = 5 * 5 * 5
X = 128  # partitions (dim 2 of input)
ZSLAB = 32  # output z rows per core
ZROWS = ZSLAB + 2 * RADIUS  # z rows incl halo
BLK = 16  # z rows per PSUM block
NBLK = ZSLAB // BLK

MODE = os.environ.get("BILAT_MODE", "f16")  # "f16" or "f32"
TRACE = bool(int(os.environ.get("BILAT_TRACE", "0")))

LAST_RESULTS = None  # BassKernelResults of most recent run (for test.py)

_TAPS = [(dx, dy, dz)
         for dx in range(-RADIUS, RADIUS + 1)
         for dy in range(-RADIUS, RADIUS + 1)
         for dz in range(-RADIUS, RADIUS + 1)]

_PROG_CACHE = {}


def _build_program(mode):
    f32 = mybir.dt.float32
    f32r = mybir.dt.float32r
    f16 = mybir.dt.float16
    if mode == "f16":
        dt_x, dt_wp, dt_id, nv, wid = f16, f16, f16, 10, 136
    else:
        dt_x, dt_wp, dt_id, nv, wid = f32, f32r, f32r, 5, 132
    np_x = mybir.dt.np(dt_x)

    nc = bacc.Bacc("TRN2", target_bir_lowering=False, debug=False, num_devices=8)
    xs = nc.dram_tensor("xs", [X, nv * ZROWS, wid], dt_x, kind="ExternalInput")
    cb = nc.dram_tensor("cb", [X, NTAPS + 1], f32, kind="ExternalInput")
    ident = nc.dram_tensor("ident", [X, X], f32, kind="ExternalInput")
    out = nc.dram_tensor("out", [X, ZSLAB * 128], f32, kind="ExternalOutput")

    Sq = mybir.ActivationFunctionType.Square
    Ex = mybir.ActivationFunctionType.Exp

    nb = 3 if mode == "f16" else 2
    with TileContext(nc) as tc:
        with (
            tc.tile_pool(name="big", bufs=1) as bigpool,
            tc.tile_pool(name="dd", bufs=nb) as dpool,
            tc.tile_pool(name="ss", bufs=nb) as spool,
            tc.tile_pool(name="ww", bufs=nb) as wpool,
            tc.tile_pool(name="pp", bufs=nb) as ppool,
            tc.tile_pool(name="ev", bufs=1) as epool,
            tc.tile_pool(name="ps", bufs=1, space="PSUM") as psp,
        ):
            xs_t = bigpool.tile([X, nv * ZROWS, wid], dt_x)
            nc.sync.dma_start(out=xs_t, in_=xs.ap())
            cb_t = bigpool.tile([X, NTAPS + 1], f32)
            nc.sync.dma_start(out=cb_t, in_=cb.ap())
            id_f32 = bigpool.tile([X, X], f32)
            nc.sync.dma_start(out=id_f32, in_=ident.ap())
            id_t = bigpool.tile([X, X], dt_id)
            nc.vector.tensor_copy(out=id_t, in_=id_f32)

            def read_ap(dx, dy, dz, blk):
                # AP into xs_t for tap (dx,dy,dz), z-block blk: [128,BLK,128]
                if mode == "f16":
                    v = (dx + RADIUS) * 2 + (dy & 1)
                    col0 = 2 + dy + (dy & 1)
                else:
                    v = dx + RADIUS
                    col0 = 2 + dy
                r0 = v * ZROWS + RADIUS + dz + BLK * blk
                return xs_t[:, r0 : r0 + BLK, col0 : col0 + 128]

            for blk in range(NBLK):
                p_num = psp.tile([X, BLK, 128], mybir.dt.float32, tag="num")
                p_den = psp.tile([X, BLK, 128], mybir.dt.float32, tag="den")
                for k, (dx, dy, dz) in enumerate(_TAPS):
                    first = k == 0
                    last = k == NTAPS - 1
                    base = read_ap(0, 0, 0, blk)
                    shft = read_ap(dx, dy, dz, blk)
                    d_t = dpool.tile([X, BLK, 128], dt_x)
                    nc.vector.tensor_sub(out=d_t, in0=base, in1=shft)
                    s_t = spool.tile([X, BLK, 128], mybir.dt.float32)
                    nc.scalar.activation(s_t, d_t, Sq)
                    w_t = wpool.tile([X, BLK, 128], dt_wp)
                    nc.scalar.activation(
                        w_t, s_t, Ex,
                        bias=cb_t[:, k : k + 1],
                        scale=cb_t[:, NTAPS : NTAPS + 1],
                    )
                    p_t = ppool.tile([X, BLK, 128], dt_wp)
                    nc.vector.tensor_mul(out=p_t, in0=w_t, in1=shft)
                    for r in range(BLK // 4):
                        nc.tensor.matmul(
                            p_num[:, 4 * r : 4 * r + 4, :], id_t,
                            p_t[:, 4 * r : 4 * r + 4, :],
                            start=first, stop=last,
                        )
                        nc.tensor.matmul(
                            p_den[:, 4 * r : 4 * r + 4, :], id_t,
                            w_t[:, 4 * r : 4 * r + 4, :],
                            start=first, stop=last,
                        )
                rec_t = epool.tile([X, BLK, 128], mybir.dt.float32, tag="rec")
                nc.vector.reciprocal(out=rec_t, in_=p_den)
                o_t = epool.tile([X, BLK, 128], mybir.dt.float32, tag="out")
                nc.vector.tensor_mul(out=o_t, in0=p_num, in1=rec_t)
                nc.sync.dma_start(
                    out=out.ap()[:, BLK * 128 * blk : BLK * 128 * (blk + 1)],
                    in_=o_t,
                )
    nc.compile()
    return nc, np_x


def _prep_core_inputs(vol, z0, big, np_x, mode):
    """vol: (128,128,128) f32 volume (x,y,z) for one batch. Returns xs array."""
    nv = 10 if mode == "f16" else 5
    wid = 136 if mode == "f16" else 132
    slab = np.full((X, ZROWS, 130), big, np.float32)
    zlo = z0 - RADIUS
    zs_lo, zs_hi = max(0, zlo), min(128, z0 + ZSLAB + RADIUS)
    # rows (z_local) x cols (y)
    slab[:, zs_lo - zlo : zs_hi - zlo, 2:130] = vol[:, :, zs_lo:zs_hi].transpose(0, 2, 1)
    xs = np.full((X, nv, ZROWS, wid), big, np_x)
    for dx in range(-RADIUS, RADIUS + 1):
        var = np.full((X, ZROWS, 130), big, np.float32)
        if dx >= 0:
            var[: X - dx] = slab[dx:]
        else:
            var[-dx:] = slab[: X + dx]
        if mode == "f16":
            v = (dx + RADIUS) * 2
            xs[:, v, :, 0:130] = var  # parity 0: y_real at col 2
            xs[:, v + 1, :, 1:131] = var  # parity 1: y_real at col 3
        else:
            xs[:, dx + RADIUS, :, 0:130] = var
    return xs.reshape(X, nv * ZROWS, wid)


def kernel(input_img, sigma_x, sigma_y, sigma_z, color_sigma):
    global LAST_RESULTS
    img = np.asarray(input_img, dtype=np.float32)
    B = img.shape[0]
    sx = float(np.asarray(sigma_x))
    sy = float(np.asarray(sigma_y))
    sz = float(np.asarray(sigma_z))
    cs = float(np.asarray(color_sigma))
    c = 1.0 / (2.0 * cs * cs)

    if MODE not in _PROG_CACHE:
        _PROG_CACHE[MODE] = _build_program(MODE)
    nc, np_x = _PROG_CACHE[MODE]

    # per-tap log spatial weights and exp scale
    cbv = np.zeros((X, NTAPS + 1), np.float32)
    for k, (dx, dy, dz) in enumerate(_TAPS):
        cbv[:, k] = -(dx * dx / (2 * sx * sx) + dy * dy / (2 * sy * sy)
                      + dz * dz / (2 * sz * sz))
    cbv[:, NTAPS] = -c

    xmax = float(np.abs(img).max())
    big = xmax + np.sqrt(95.0 / c)

    eye = np.eye(X, dtype=np.float32)
    in_maps = []
    for core in range(8):
        b, q = divmod(core, 4)
        xs = _prep_core_inputs(img[b, 0], q * ZSLAB, big, np_x, MODE)
        in_maps.append({"xs": xs, "cb": cbv, "ident": eye})

    res = bass_utils.run_bass_kernel_spmd(
        nc, in_maps, core_ids=list(range(8)), trace=TRACE
    )
    LAST_RESULTS = res

    outv = np.empty_like(img)
    for core in range(8):
        b, q = divmod(core, 4)
        o = res.results[core]["out"].reshape(X, ZSLAB, 128)  # (x, z_local, y)
        outv[b, 0, :, :, q * ZSLAB : (q + 1) * ZSLAB] = o.transpose(0, 2, 1)
    return outv


# revision 3
# speedup vs baseline: 1.1904x; 1.1904x over previous
"""3D bilateral filter (RADIUS=2, 5x5x5 window) on 8 Trainium2 NeuronCores.

Sharding: 8 cores = 2 batches x 4 z-slabs of 32 (halo 2 handled host-side).
Per-core layout: partitions = x (128), free dim = (z_local rows) x (padded y).
Out-of-volume taps are neutralized by padding with a large value BIG chosen so
the range weight exp(-c*(x-BIG)^2 + b) underflows to exactly 0 on the ACT LUT.
x-axis tap shifts are pre-materialized host-side as 5 shifted variants (plus a
second y-parity copy in fp16 mode, keeping DVE reads 4B-aligned for 2x mode).

Per tap on-chip:  D = x - x_shift (DVE), S = D^2 (ACT Square), W = exp(-c*S+b)
(ACT Exp, b = log spatial weight), P = W * x_shift (DVE), then num += P and
den += W via identity-matmul accumulation into PSUM (PE does all adds).
Finally out = num * reciprocal(den) (DVE) and DMA out.
"""

import os
import sys

import numpy as np

for _p in ("/root/.axon_site", "/root/.axon_site/_ro/trn_rl_repo",
           "/root/.axon_site/_ro/pypackages", "/opt/trn_rl_repo"):
    if os.path.isdir(_p) and _p not in sys.path:
        sys.path.append(_p)

import concourse.bacc as bacc
import concourse.mybir as mybir
from concourse.tile import TileContext
from concourse import bass_utils

RADIUS = 2
NTAPS = 5 * 5 * 5
X = 128  # partitions (dim 2 of input)
ZSLAB = 32  # output z rows per core
ZROWS = ZSLAB + 2 * RADIUS  # z rows incl halo
BLK = 16  # z rows per PSUM block
NBLK = ZSLAB // BLK

MODE = os.environ.get("BILAT_MODE", "f16")  # "f16" or "f32"
TRACE = bool(int(os.environ.get("BILAT_TRACE", "0")))

LAST_RESULTS = None  # BassKernelResults of most recent run (for test.py)

_TAPS = [(dx, dy, dz)
         for dx in range(-RADIUS, RADIUS + 1)
         for dy in range(-RADIUS, RADIUS + 1)
         for dz in range(-RADIUS, RADIUS + 1)]

_PROG_CACHE = {}


def _build_program(mode):
    f32 = mybir.dt.float32
    f32r = mybir.dt.float32r
    f16 = mybir.dt.float16
    if mode == "f16":
        dt_x, dt_wp, dt_id, nv, wid = f16, f16, f16, 10, 136
    else:
        dt_x, dt_wp, dt_id, nv, wid = f32, f32r, f32r, 5, 132
    np_x = mybir.dt.np(dt_x)

    nc = bacc.Bacc("TRN2", target_bir_lowering=False, debug=False, num_devices=8)
    xs = nc.dram_tensor("xs", [X, nv * ZROWS, wid], dt_x, kind="ExternalInput")
    cb = nc.dram_tensor("cb", [X, NTAPS + 1], f32, kind="ExternalInput")
    ident = nc.dram_tensor("ident", [X, X], f32, kind="ExternalInput")
    out = nc.dram_tensor("out", [X, ZSLAB * 128], f32, kind="ExternalOutput")

    Sq = mybir.ActivationFunctionType.Square
    Ex = mybir.ActivationFunctionType.Exp

    nb = 3 if mode == "f16" else 2
    with TileContext(nc) as tc:
        with (
            tc.tile_pool(name="big", bufs=1) as bigpool,
            tc.tile_pool(name="dd", bufs=nb) as dpool,
            tc.tile_pool(name="ss", bufs=nb) as spool,
            tc.tile_pool(name="ww", bufs=nb) as wpool,
            tc.tile_pool(name="pp", bufs=nb) as ppool,
            tc.tile_pool(name="ev", bufs=1) as epool,
            tc.tile_pool(name="ps", bufs=1, space="PSUM") as psp,
        ):
            xs_t = bigpool.tile([X, nv * ZROWS, wid], dt_x)
            nc.sync.dma_start(out=xs_t, in_=xs.ap())
            cb_t = bigpool.tile([X, NTAPS + 1], f32)
            nc.sync.dma_start(out=cb_t, in_=cb.ap())
            id_f32 = bigpool.tile([X, X], f32)
            nc.sync.dma_start(out=id_f32, in_=ident.ap())
            id_t = bigpool.tile([X, X], dt_id)
            nc.vector.tensor_copy(out=id_t, in_=id_f32)

            def read_ap(dx, dy, dz, blk):
                # AP into xs_t for tap (dx,dy,dz), z-block blk: [128,BLK,128]
                if mode == "f16":
                    v = (dx + RADIUS) * 2 + (dy & 1)
                    col0 = 2 + dy + (dy & 1)
                else:
                    v = dx + RADIUS
                    col0 = 2 + dy
                r0 = v * ZROWS + RADIUS + dz + BLK * blk
                return xs_t[:, r0 : r0 + BLK, col0 : col0 + 128]

            for blk in range(NBLK):
                p_num = psp.tile([X, BLK, 128], mybir.dt.float32, tag="num")
                p_den = psp.tile([X, BLK, 128], mybir.dt.float32, tag="den")
                for k, (dx, dy, dz) in enumerate(_TAPS):
                    first = k == 0
                    last = k == NTAPS - 1
                    base = read_ap(0, 0, 0, blk)
                    shft = read_ap(dx, dy, dz, blk)
                    d_t = dpool.tile([X, BLK, 128], dt_x)
                    nc.vector.tensor_sub(out=d_t, in0=base, in1=shft)
                    # Balance the square op between DVE (fp16 2x) and ACT
                    sq_on_dve = mode == "f16" and (k % 12) < 5
                    if sq_on_dve:
                        s_t = spool.tile([X, BLK, 128], dt_x, tag="s16")
                        nc.vector.tensor_mul(out=s_t, in0=d_t, in1=d_t)
                    else:
                        s_t = spool.tile([X, BLK, 128], mybir.dt.float32, tag="s32")
                        nc.scalar.activation(s_t, d_t, Sq)
                    w_t = wpool.tile([X, BLK, 128], dt_wp)
                    nc.scalar.activation(
                        w_t, s_t, Ex,
                        bias=cb_t[:, k : k + 1],
                        scale=cb_t[:, NTAPS : NTAPS + 1],
                    )
                    p_t = ppool.tile([X, BLK, 128], dt_wp)
                    nc.vector.tensor_mul(out=p_t, in0=w_t, in1=shft)
                    for r in range(BLK // 4):
                        nc.tensor.matmul(
                            p_num[:, 4 * r : 4 * r + 4, :], id_t,
                            p_t[:, 4 * r : 4 * r + 4, :],
                            start=first, stop=last,
                        )
                        nc.tensor.matmul(
                            p_den[:, 4 * r : 4 * r + 4, :], id_t,
                            w_t[:, 4 * r : 4 * r + 4, :],
                            start=first, stop=last,
                        )
                rec_t = epool.tile([X, BLK, 128], mybir.dt.float32, tag="rec")
                nc.vector.reciprocal(out=rec_t, in_=p_den)
                o_t = epool.tile([X, BLK, 128], mybir.dt.float32, tag="out")
                nc.vector.tensor_mul(out=o_t, in0=p_num, in1=rec_t)
                nc.sync.dma_start(
                    out=out.ap()[:, BLK * 128 * blk : BLK * 128 * (blk + 1)],
                    in_=o_t,
                )
    nc.compile()
    return nc, np_x


def _prep_core_inputs(vol, z0, big, np_x, mode):
    """vol: (128,128,128) f32 volume (x,y,z) for one batch. Returns xs array."""
    nv = 10 if mode == "f16" else 5
    wid = 136 if mode == "f16" else 132
    slab = np.full((X, ZROWS, 130), big, np.float32)
    zlo = z0 - RADIUS
    zs_lo, zs_hi = max(0, zlo), min(128, z0 + ZSLAB + RADIUS)
    # rows (z_local) x cols (y)
    slab[:, zs_lo - zlo : zs_hi - zlo, 2:130] = vol[:, :, zs_lo:zs_hi].transpose(0, 2, 1)
    xs = np.full((X, nv, ZROWS, wid), big, np_x)
    for dx in range(-RADIUS, RADIUS + 1):
        var = np.full((X, ZROWS, 130), big, np.float32)
        if dx >= 0:
            var[: X - dx] = slab[dx:]
        else:
            var[-dx:] = slab[: X + dx]
        if mode == "f16":
            v = (dx + RADIUS) * 2
            xs[:, v, :, 0:130] = var  # parity 0: y_real at col 2
            xs[:, v + 1, :, 1:131] = var  # parity 1: y_real at col 3
        else:
            xs[:, dx + RADIUS, :, 0:130] = var
    return xs.reshape(X, nv * ZROWS, wid)


def kernel(input_img, sigma_x, sigma_y, sigma_z, color_sigma):
    global LAST_RESULTS
    img = np.asarray(input_img, dtype=np.float32)
    B = img.shape[0]
    sx = float(np.asarray(sigma_x))
    sy = float(np.asarray(sigma_y))
    sz = float(np.asarray(sigma_z))
    cs = float(np.asarray(color_sigma))
    c = 1.0 / (2.0 * cs * cs)

    if MODE not in _PROG_CACHE:
        _PROG_CACHE[MODE] = _build_program(MODE)
    nc, np_x = _PROG_CACHE[MODE]

    # per-tap log spatial weights and exp scale
    cbv = np.zeros((X, NTAPS + 1), np.float32)
    for k, (dx, dy, dz) in enumerate(_TAPS):
        cbv[:, k] = -(dx * dx / (2 * sx * sx) + dy * dy / (2 * sy * sy)
                      + dz * dz / (2 * sz * sz))
    cbv[:, NTAPS] = -c

    xmax = float(np.abs(img).max())
    big = xmax + np.sqrt(95.0 / c)

    eye = np.eye(X, dtype=np.float32)
    in_maps = []
    for core in range(8):
        b, q = divmod(core, 4)
        xs = _prep_core_inputs(img[b, 0], q * ZSLAB, big, np_x, MODE)
        in_maps.append({"xs": xs, "cb": cbv, "ident": eye})

    res = bass_utils.run_bass_kernel_spmd(
        nc, in_maps, core_ids=list(range(8)), trace=TRACE
    )
    LAST_RESULTS = res

    outv = np.empty_like(img)
    for core in range(8):
        b, q = divmod(core, 4)
        o = res.results[core]["out"].reshape(X, ZSLAB, 128)  # (x, z_local, y)
        outv[b, 0, :, :, q * ZSLAB : (q + 1) * ZSLAB] = o.transpose(0, 2, 1)
    return outv
